# revision 24
# baseline (speedup 1.0000x reference)
"""Multi-head attention (B=64, N=577, E=1024, H=16) on 8 TRN2 NeuronCores.

Strategy: pure data-parallel over batch (8 batches/core), full weights on
every core. The host pre-transposes and pre-casts x -> x^T and W -> W^T in
bf16, so the device does no transposes at all: per batch it computes
Q^T/K^T = W^T-tiles @ x^T and V = x^T-tiles @ Wv^T directly in bf16
(full PE rate + fast weight load). Per (batch, head-pair): scores are
computed in transposed orientation S^T[nk, nq] (softmax needs no
probability transpose); both heads of a pair write one 2-bank PSUM tile so
a single ACT exp covers them; the softmax denominator comes free from a
ones-column appended to V (V tiles padded to 128 cols for FWL); PV
accumulates [d(+sum), nq] per head and results are stored as
[b, head, d+1, n] with normalization + final permute on the host.
Softmax skips max-subtraction (scores ~ N(0,1); exp cannot overflow).
"""

import numpy as np

B, N, E, H, D = 64, 577, 1024, 16, 64
NCORES = 8
BL = B // NCORES            # batches per core
NP = 578                    # padded nq (zero col 577)
EB = E // 128               # 8 e-blocks
NBL = [(i * 128, min(128, N - i * 128)) for i in range((N + 127) // 128)]
CHUNKS = [(0, 290), (290, 288)]  # nq chunks (psum bank holds <=512 f32)
# key-block pairs per (j, chunk): [(0,1), (2,3), (4,)]
IGRP = [(0, 1), (2, 3), (4,)]

_CACHE = {}


def _build(cfg=None):
    cfg = cfg or {}
    ST = cfg.get("st", 2)       # 2-bank score-group psum bufs
    PV = cfg.get("pv", 2)       # pO psum bufs
    MM = cfg.get("mm", 2)       # projection psum bufs
    ES = cfg.get("es", 6)       # es sbuf bufs
    FILL = cfg.get("fill", 3)   # filler steps per igroup
    PSDMA = cfg.get("psdma", False)  # DMA out directly from psum
    GRP = cfg.get("grp", True)  # 2-head grouped exp (2-bank pS)
    LAG = cfg.get("lag", 2)     # PV trails S^T/exp by LAG igroups
    import concourse.mybir as mybir
    import concourse.tile as tile
    from concourse import bacc

    f32 = mybir.dt.float32
    bf16 = mybir.dt.bfloat16
    Exp = mybir.ActivationFunctionType.Exp

    nc = bacc.Bacc("TRN2", target_bir_lowering=False, debug=False,
                   num_devices=NCORES)
    # host-prepped layouts (bf16): x^T and W^T with partition dim explicit
    xin = nc.declare_dram_parameter("xt", [BL, 128, EB, NP], bf16,
                                    isOutput=False)
    win = nc.declare_dram_parameter("wt", [128, 3, EB, E], bf16,
                                    isOutput=False)
    out = nc.declare_dram_parameter("out", [BL, H, D + 1, N], bf16,
                                    isOutput=True)

    with tile.TileContext(nc) as tc:
        with (
            tc.tile_pool(name="sb", bufs=1) as sb,
            tc.tile_pool(name="ps", bufs=1, space="PSUM") as ps,
        ):
            ones16 = sb.tile([128, H, 1], bf16, tag="ones", name="ones16")
            nc.gpsimd.memset(ones16[:], 1.0)

            # all weights, one DMA: [p, wi, ei, eout]
            wt = sb.tile([128, 3, EB, E], bf16, tag="wt", name="wt")
            nc.sync.dma_start(out=wt[:], in_=win[:])

            def p1(b):
                """Per-batch projections: x^T DMA, Q^T, K^T, V(+ones).

                Yields between PE ops so it can run as a filler inside the
                previous batch's attention phase. Returns (xt, qt, kt, vext)
                tiles; caller must fully drain before using them.
                """
                xt = sb.tile([128, EB, NP], bf16, tag="xt", bufs=2,
                             name="xt")
                nc.sync.dma_start(out=xt[:], in_=xin[b])
                qt = sb.tile([128, EB, NP], bf16, tag="qt", bufs=2,
                             name="qt")
                kt = sb.tile([128, EB, NP], bf16, tag="kt", bufs=2,
                             name="kt")
                vext = []
                for nb in range(len(NBL)):
                    vx = sb.tile([128, H, 128], bf16, tag=f"vx_{nb}",
                                 bufs=2, name=f"vx{nb}")
                    vext.append(vx)

                def gen():
                    # Q^T / K^T: per (eo-block, chunk): 8 accumulating mms
                    for wi, dst in ((0, qt), (1, kt)):
                        for eo in range(EB):
                            for c0, cw in CHUNKS:
                                pq = ps.tile([128, 512], f32, tag="mm",
                                             bufs=MM, name="pq")
                                for ei in range(EB):
                                    nc.tensor.matmul(
                                        pq[:, :cw],
                                        wt[:, wi, ei, eo * 128:(eo + 1) * 128],
                                        xt[:, ei, c0:c0 + cw],
                                        start=(ei == 0), stop=(ei == EB - 1))
                                    yield None
                                nc.vector.tensor_copy(
                                    dst[:, eo, c0:c0 + cw], pq[:, :cw])
                                yield None
                    # V -> vext (+ ones col); pad cols D+1.. stay stale
                    # (only rows/cols we never read feed from them)
                    for nb, (n0, nsz) in enumerate(NBL):
                        nc.vector.tensor_copy(
                            vext[nb][:nsz, :, D:D + 1], ones16[:nsz, :, :])
                        for ec in range(2):
                            pv = ps.tile([128, 512], f32, tag="mm", bufs=MM,
                                         name="pv")
                            for ei in range(EB):
                                nc.tensor.matmul(
                                    pv[:nsz, :], xt[:, ei, n0:n0 + nsz],
                                    wt[:, 2, ei, ec * 512:(ec + 1) * 512],
                                    start=(ei == 0), stop=(ei == EB - 1))
                                yield None
                            nc.vector.tensor_copy(
                                vext[nb][:nsz, ec * 8:(ec + 1) * 8, 0:D],
                                pv[:nsz, :].rearrange("p (h d) -> p h d",
                                                      d=D))
                            yield None

                return xt, qt, kt, vext, gen()

            fillers = []

            def fill(n):
                for _ in range(n):
                    if not fillers:
                        return
                    for it in list(fillers):
                        if next(it, StopIteration) is StopIteration:
                            fillers.remove(it)
                        else:
                            break

            def p2(b, qt, kt, vext):
                """Attention for batch b from SBUF-resident Q^T/K^T/V."""
                for j in range(H // 2):
                    for ci, (c0, cw) in enumerate(CHUNKS):
                        pO = [ps.tile([128, 290], f32, tag="pv", bufs=PV,
                                      name=f"pO{h}") for h in range(2)]
                        es = []          # es[i] = [128, 2, 290] bf16
                        ngrp = len(NBL)
                        for i in range(ngrp + LAG):
                            if i < ngrp:
                                k0, ksz = NBL[i]
                                if GRP:
                                    # both heads -> one 2-bank psum tile
                                    pS = ps.tile([128, 2, 512], f32,
                                                 tag="st", bufs=ST,
                                                 name="pS")
                                    pSh = [pS[:ksz, h, :cw] for h in (0, 1)]
                                else:
                                    pS2 = [ps.tile([128, 512], f32,
                                                   tag="st", bufs=2 * ST,
                                                   name="pS")
                                           for h in (0, 1)]
                                    pSh = [p[:ksz, :cw] for p in pS2]
                                for h in range(2):
                                    nc.tensor.matmul(
                                        pSh[h],
                                        kt[h * 64:h * 64 + 64, j,
                                           k0:k0 + ksz],
                                        qt[h * 64:h * 64 + 64, j,
                                           c0:c0 + cw],
                                        start=True, stop=True,
                                        tile_position=(h * 64, 0))
                                e = sb.tile([128, 2, 290], bf16, tag="es",
                                            bufs=ES, name="es")
                                if GRP:
                                    nc.scalar.activation(
                                        e[:ksz, :, :cw], pS[:ksz, :, :cw],
                                        Exp, scale=0.125)
                                else:
                                    for h in range(2):
                                        nc.scalar.activation(
                                            e[:ksz, h, :cw], pSh[h], Exp,
                                            scale=0.125)
                                es.append(e)
                            if i >= LAG:
                                kp, kpsz = NBL[i - LAG]
                                for h in range(2):
                                    nc.tensor.matmul(
                                        pO[h][:, :cw],
                                        vext[i - LAG][:kpsz, 2 * j + h, :],
                                        es[i - LAG][:kpsz, h, :cw],
                                        start=(i == LAG),
                                        stop=(i == ngrp + LAG - 1))
                            fill(FILL)
                        # store both heads
                        cwo = min(cw, N - c0)
                        if PSDMA:
                            for h in range(2):
                                eng = nc.sync if h == 0 else nc.gpsimd
                                eng.dma_start(
                                    out=out[b, 2 * j + h, :, c0:c0 + cwo],
                                    in_=pO[h][:D + 1, :cwo])
                                fill(2)
                        else:
                            ov = sb.tile([D + 1, 2, 290], bf16, tag="ov",
                                         bufs=3, name="ov")
                            for h in range(2):
                                nc.vector.tensor_copy(ov[:, h, :cw],
                                                      pO[h][:D + 1, :cw])
                                fill(2)
                            eng = (nc.sync if (2 * j + ci) % 2 == 0
                                   else nc.gpsimd)
                            eng.dma_start(
                                out=out[b, 2 * j:2 * j + 2, :, c0:c0 + cwo]
                                    .rearrange("h d n -> d h n"),
                                in_=ov[:, :, :cwo])
                            fill(2)

            xt, qt, kt, vext, g = p1(0)
            fillers.append(g)
            fill(10 ** 6)
            for b in range(BL):
                if b + 1 < BL:
                    nxt = p1(b + 1)
                    fillers.append(nxt[4])
                p2(b, qt, kt, vext)
                if b + 1 < BL:
                    fill(10 ** 6)
                    xt, qt, kt, vext = nxt[:4]
            fill(10 ** 6)

    nc.compile()
    return nc


def _prep(x, Wq, Wk, Wv):
    import ml_dtypes
    bf16 = ml_dtypes.bfloat16
    x = np.asarray(x, dtype=np.float32)
    xs = x.reshape(NCORES, BL, N, E)
    # [c, b, n, (ei p)] -> [c, b, p, ei, n], pad n to NP
    xt = np.zeros((NCORES, BL, 128, EB, NP), dtype=bf16)
    xt[..., :N] = xs.reshape(NCORES, BL, N, EB, 128).transpose(0, 1, 4, 3, 2)
    # W^T: [p, wi, ei, eo] = W_wi[eo, ei*128+p]
    ws = np.stack([np.asarray(w, dtype=np.float32) for w in (Wq, Wk, Wv)])
    wt = np.ascontiguousarray(
        ws.transpose(2, 0, 1).reshape(EB, 128, 3, E).transpose(1, 2, 0, 3)
    ).astype(bf16)
    return xt, wt


def kernel(x, Wq, Wk, Wv):
    from concourse.bass_utils import run_bass_kernel_spmd

    if "nc" not in _CACHE:
        _CACHE["nc"] = _build()
    nc = _CACHE["nc"]

    xt, wt = _prep(x, Wq, Wk, Wv)
    in_maps = [
        {"xt": np.ascontiguousarray(xt[i]), "wt": wt}
        for i in range(NCORES)
    ]
    res = run_bass_kernel_spmd(nc, in_maps, core_ids=list(range(NCORES)))
    # device emits [b, head, d(+sums), n]; normalize + permute on the host
    ot = np.concatenate(
        [np.asarray(res.results[i]["out"], dtype=np.float32)
         for i in range(NCORES)], axis=0)
    o = ot[:, :, :D, :] / ot[:, :, D:D + 1, :]
    return np.ascontiguousarray(
        o.transpose(0, 3, 1, 2).reshape(B, N, E).astype(np.float32))
